# revision 5
# baseline (speedup 1.0000x reference)
"""Trainium2 Bass kernel for nn_Attn_61366492725428 (masked attention pooling).

Reference computation:
    hid = transpose(hidden,(1,0,2)).reshape(B,-1)          # (B, 1024)
    e   = enc @ We + (hid @ Wh)[:,None] + b                # (B, T)
    e   = e * mask
    a   = softmax(e, axis=1) * mask;  a /= a.sum(1)
    ctx = einsum('bt,bth->bh', a, enc)                     # (B, 1024)

Key identity: the per-batch constant c = hid@Wh + b shifts every *valid*
energy equally, masked entries are zeroed in both the numerator and the
renormalization denominator, and softmax's own Z cancels under the
renormalize — so exp(c) cancels exactly and the output does not depend on
hidden/Wh/b at all:
    ctx[b] = sum_t mask*exp(e_enc) * enc / sum_t mask*exp(e_enc)
(verified vs the jax reference: rel err ~2e-6, pure fp noise).

Sharding: batch B=32 across 8 cores (4 batches/core); We replicated.
"""

import sys
import numpy as np

N_CORES = 8
B, T, HE = 32, 2048, 1024
B_LOC = B // N_CORES          # 4 batches per core
TT = 128                      # t-tile (partition dim)
NT = T // TT                  # 16 t-tiles per batch
NH = 512                      # matmul free-dim limit (one PSUM bank of f32)

_CACHE = {}


def _build_nc():
    import concourse.bacc as bacc
    import concourse.tile as tile
    from concourse import mybir

    f32 = mybir.dt.float32

    nc = bacc.Bacc("TRN2")
    enc = nc.dram_tensor("enc", [B_LOC, T, HE], f32, kind="ExternalInput")
    msk = nc.dram_tensor("msk", [B_LOC, T], f32, kind="ExternalInput")
    we = nc.dram_tensor("we", [1, HE], f32, kind="ExternalInput")
    out = nc.dram_tensor("out", [B_LOC, HE], f32, kind="ExternalOutput")

    with tile.TileContext(nc) as tc:
        with (
            tc.tile_pool(name="singles", bufs=1) as singles,
            tc.tile_pool(name="encp", bufs=24) as encp,
            tc.tile_pool(name="scratch", bufs=2) as scratchp,
            tc.tile_pool(name="stats", bufs=4) as stats,
            tc.tile_pool(name="ctxp", bufs=2, space="PSUM") as ctxp,
            tc.tile_pool(name="spsum", bufs=2, space="PSUM") as spsum,
        ):
            # We broadcast to all 128 partitions (one-time, 512KB)
            we_b = singles.tile([128, HE], f32, tag="we_b")
            nc.gpsimd.dma_start(out=we_b, in_=we[0:1, :].partition_broadcast(128))

            ones_col = singles.tile([128, 1], f32, tag="ones")
            nc.vector.memset(ones_col, 1.0)

            # mask, transposed to [t-within-tile, tile] layout, per batch
            mask_sb = []
            for b in range(B_LOC):
                mb = singles.tile([128, NT], f32, tag=f"mask{b}")
                nc.gpsimd.dma_start(
                    out=mb,
                    in_=msk[b : b + 1, :].rearrange("o (j p) -> (o p) j", p=TT),
                )
                mask_sb.append(mb)

            for b in range(B_LOC):
                e_b = stats.tile([128, NT], f32, tag="e_b")
                enc_tiles = []
                for j in range(NT):
                    et = encp.tile([128, HE], f32, tag="enc_t")
                    nc.sync.dma_start(out=et, in_=enc[b, j * TT : (j + 1) * TT, :])
                    enc_tiles.append(et)
                    sc = scratchp.tile([128, HE], f32, tag="sc")
                    # e_b[:, j] = sum_h et * We   (mul on DVE, free-dim sum on ACT)
                    nc.vector.tensor_mul(sc, et, we_b)
                    nc.scalar.activation(
                        sc,
                        sc,
                        mybir.ActivationFunctionType.Copy,
                        accum_out=e_b[:, j : j + 1],
                    )

                # w = mask * exp(e * mask); ws[p] = sum_j w[p, j]
                masked = stats.tile([128, NT], f32, tag="masked")
                nc.vector.tensor_mul(masked, e_b, mask_sb[b])
                expd = stats.tile([128, NT], f32, tag="expd")
                nc.scalar.activation(expd, masked, mybir.ActivationFunctionType.Exp)
                w_b = stats.tile([128, NT], f32, tag="w_b")
                ws = stats.tile([128, 1], f32, tag="ws")
                nc.vector.tensor_mul(w_b, expd, mask_sb[b])
                nc.vector.reduce_sum(ws, w_b, axis=mybir.AxisListType.X)

                # S = sum_p ws[p]  (partition reduce via PE), recip = 1/S
                s_ps = spsum.tile([1, 1], f32, tag="s_ps")
                nc.tensor.matmul(s_ps, ws, ones_col, start=True, stop=True)
                recip = stats.tile([1, 1], f32, tag="recip")
                nc.vector.reciprocal(recip, s_ps)

                # ctx[h] = sum_t w[t] * enc[t, h], accumulated over t-tiles
                ctx = ctxp.tile([1, 2, NH], f32, tag="ctx")
                for j in range(NT):
                    for h in range(2):
                        nc.tensor.matmul(
                            ctx[:, h, :],
                            w_b[:, j : j + 1],
                            enc_tiles[j][:, h * NH : (h + 1) * NH],
                            start=(j == 0),
                            stop=(j == NT - 1),
                        )

                # out[b] = ctx * (1/S)
                ctx_sb = stats.tile([1, HE], f32, tag="ctx_sb")
                for h in range(2):
                    nc.scalar.mul(
                        out=ctx_sb[:, h * NH : (h + 1) * NH],
                        in_=ctx[:, h, :],
                        mul=recip,
                    )
                nc.gpsimd.dma_start(out=out[b : b + 1, :], in_=ctx_sb)

    nc.compile()
    return nc


def _get_nc():
    if "nc" not in _CACHE:
        _CACHE["nc"] = _build_nc()
    return _CACHE["nc"]


def kernel(hidden, encoder_outputs, mask, W, b):
    from concourse import bass_utils

    # avoid S3 upload attempts if tracing is enabled
    bass_utils.upload_artifacts = lambda tmpdir: f"local:{tmpdir}"

    nc = _get_nc()
    enc = np.ascontiguousarray(np.asarray(encoder_outputs, dtype=np.float32))
    msk = np.ascontiguousarray(np.asarray(mask, dtype=np.float32))
    we = np.ascontiguousarray(np.asarray(W, dtype=np.float32)[0:1, HE:])

    in_maps = []
    for i in range(N_CORES):
        in_maps.append(
            {
                "enc": np.ascontiguousarray(enc[i * B_LOC : (i + 1) * B_LOC]),
                "msk": np.ascontiguousarray(msk[i * B_LOC : (i + 1) * B_LOC]),
                "we": we,
            }
        )
    res = bass_utils.run_bass_kernel_spmd(nc, in_maps, core_ids=list(range(N_CORES)))
    _CACHE["last_results"] = res
    return np.concatenate([r["out"] for r in res.results], axis=0)


# revision 6
# speedup vs baseline: 1.2332x; 1.2332x over previous
"""Trainium2 Bass kernel for nn_Attn_61366492725428 (masked attention pooling).

Reference computation:
    hid = transpose(hidden,(1,0,2)).reshape(B,-1)          # (B, 1024)
    e   = enc @ We + (hid @ Wh)[:,None] + b                # (B, T)
    e   = e * mask
    a   = softmax(e, axis=1) * mask;  a /= a.sum(1)
    ctx = einsum('bt,bth->bh', a, enc)                     # (B, 1024)

Key identity: the per-batch constant c = hid@Wh + b shifts every *valid*
energy equally, masked entries are zeroed in both the numerator and the
renormalization denominator, and softmax's own Z cancels under the
renormalize — so exp(c) cancels exactly and the output does not depend on
hidden/Wh/b at all:
    ctx[b] = sum_t mask*exp(e_enc) * enc / sum_t mask*exp(e_enc)
(verified vs the jax reference: rel err ~2e-6, pure fp noise).

Device pipeline per enc tile [128t, 1024h] (f32, natural layout):
    DVE : p16 = enc * We_bcast            (f32 in, fp16 out, one pass)
    ACT : accum-copy over p16             -> e[:, j] = sum_h p  (energies)
    PE  : ctx_psum += w[t]^T @ p16        (fp16 matmul, 1 HW pass)
and ctx = ctx_psum * (1/S) * (1/We)  — dividing the *product*-weighted sum
by We recovers the enc-weighted sum (error ~3e-4, fp16 rounding of p).

Sharding: batch B=32 across 8 cores (4 batches/core); We replicated.
"""

import sys
import numpy as np

N_CORES = 8
B, T, HE = 32, 2048, 1024
B_LOC = B // N_CORES          # 4 batches per core
TT = 128                      # t-tile (partition dim)
NT = T // TT                  # 16 t-tiles per batch
NH = 512                      # matmul free-dim limit (one PSUM bank of f32)

_CACHE = {}


def _build_nc():
    import concourse.bacc as bacc
    import concourse.tile as tile
    from concourse import mybir

    f32 = mybir.dt.float32
    f16 = mybir.dt.float16
    Copy = mybir.ActivationFunctionType.Copy
    Exp = mybir.ActivationFunctionType.Exp

    nc = bacc.Bacc("TRN2")
    enc = nc.dram_tensor("enc", [B_LOC, T, HE], f32, kind="ExternalInput")
    msk = nc.dram_tensor("msk", [B_LOC, T], f32, kind="ExternalInput")
    we = nc.dram_tensor("we", [1, HE], f32, kind="ExternalInput")
    out = nc.dram_tensor("out", [B_LOC, HE], f32, kind="ExternalOutput")

    with tile.TileContext(nc) as tc:
        with (
            tc.tile_pool(name="singles", bufs=1) as singles,
            tc.tile_pool(name="encp", bufs=8) as encp,
            tc.tile_pool(name="p16p", bufs=20) as p16p,
            tc.tile_pool(name="stats", bufs=4) as stats,
            tc.tile_pool(name="ctxp", bufs=2, space="PSUM") as ctxp,
            tc.tile_pool(name="spsum", bufs=2, space="PSUM") as spsum,
        ):
            # We broadcast to all 128 partitions (one-time, 512KB)
            we_b = singles.tile([128, HE], f32, tag="we_b")
            nc.gpsimd.dma_start(out=we_b, in_=we[0:1, :].partition_broadcast(128))

            # 1/We on partition 0, for the final un-scaling
            invwe = singles.tile([1, HE], f32, tag="invwe")
            nc.vector.reciprocal(invwe, we_b[0:1, :])

            ones_col = singles.tile([128, 1], f32, tag="ones")
            nc.vector.memset(ones_col, 1.0)

            # mask, transposed to [t-within-tile, tile] layout, per batch
            mask_sb = []
            for b in range(B_LOC):
                mb = singles.tile([128, NT], f32, tag=f"mask{b}")
                nc.gpsimd.dma_start(
                    out=mb,
                    in_=msk[b : b + 1, :].rearrange("o (j p) -> (o p) j", p=TT),
                )
                mask_sb.append(mb)

            for b in range(B_LOC):
                e_b = stats.tile([128, NT], f32, tag="e_b")
                p16_tiles = []
                for j in range(NT):
                    et = encp.tile([128, HE], f32, tag="enc_t")
                    nc.sync.dma_start(out=et, in_=enc[b, j * TT : (j + 1) * TT, :])
                    p16 = p16p.tile([128, HE], f16, tag="p16")
                    p16_tiles.append(p16)
                    # p16 = enc * We  (fp16 product; enc tile free after this)
                    nc.vector.tensor_mul(p16, et, we_b)
                    # e_b[:, j] = sum_h p16  (free-dim accumulate on ACT)
                    nc.scalar.activation(p16, p16, Copy, accum_out=e_b[:, j : j + 1])

                # w = mask * exp(e * mask); ws[p] = sum_j w[p, j]
                masked = stats.tile([128, NT], f32, tag="masked")
                nc.vector.tensor_mul(masked, e_b, mask_sb[b])
                expd = stats.tile([128, NT], f32, tag="expd")
                nc.scalar.activation(expd, masked, Exp)
                w_b = stats.tile([128, NT], f32, tag="w_b")
                ws = stats.tile([128, 1], f32, tag="ws")
                nc.vector.tensor_mul(w_b, expd, mask_sb[b])
                nc.vector.reduce_sum(ws, w_b, axis=mybir.AxisListType.X)
                w16 = stats.tile([128, NT], f16, tag="w16")
                nc.vector.tensor_copy(w16, w_b)

                # S = sum_p ws[p]  (partition reduce via PE), recip = 1/S
                s_ps = spsum.tile([1, 1], f32, tag="s_ps")
                nc.tensor.matmul(s_ps, ws, ones_col, start=True, stop=True)
                recip = stats.tile([1, 1], f32, tag="recip")
                nc.vector.reciprocal(recip, s_ps)

                # ctxP[h] = sum_t w[t] * p16[t, h], accumulated over t-tiles
                ctx = ctxp.tile([1, 2, NH], f32, tag="ctx")
                for j in range(NT):
                    for h in range(2):
                        nc.tensor.matmul(
                            ctx[:, h, :],
                            w16[:, j : j + 1],
                            p16_tiles[j][:, h * NH : (h + 1) * NH],
                            start=(j == 0),
                            stop=(j == NT - 1),
                        )

                # out[b] = ctxP * (1/S) * (1/We)
                tmp = stats.tile([1, HE], f32, tag="tmp")
                for h in range(2):
                    nc.scalar.mul(
                        out=tmp[:, h * NH : (h + 1) * NH],
                        in_=ctx[:, h, :],
                        mul=recip,
                    )
                ctx_sb = stats.tile([1, HE], f32, tag="ctx_sb")
                nc.vector.tensor_mul(ctx_sb, tmp, invwe)
                nc.gpsimd.dma_start(out=out[b : b + 1, :], in_=ctx_sb)

    nc.compile()
    return nc


def _get_nc():
    if "nc" not in _CACHE:
        _CACHE["nc"] = _build_nc()
    return _CACHE["nc"]


def kernel(hidden, encoder_outputs, mask, W, b):
    from concourse import bass_utils

    # avoid S3 upload attempts if tracing is enabled
    bass_utils.upload_artifacts = lambda tmpdir: f"local:{tmpdir}"

    nc = _get_nc()
    enc = np.ascontiguousarray(np.asarray(encoder_outputs, dtype=np.float32))
    msk = np.ascontiguousarray(np.asarray(mask, dtype=np.float32))
    we = np.ascontiguousarray(np.asarray(W, dtype=np.float32)[0:1, HE:])

    in_maps = []
    for i in range(N_CORES):
        in_maps.append(
            {
                "enc": np.ascontiguousarray(enc[i * B_LOC : (i + 1) * B_LOC]),
                "msk": np.ascontiguousarray(msk[i * B_LOC : (i + 1) * B_LOC]),
                "we": we,
            }
        )
    res = bass_utils.run_bass_kernel_spmd(nc, in_maps, core_ids=list(range(N_CORES)))
    _CACHE["last_results"] = res
    return np.concatenate([r["out"] for r in res.results], axis=0)


# revision 7
# speedup vs baseline: 1.3664x; 1.1080x over previous
"""Trainium2 Bass kernel for nn_Attn_61366492725428 (masked attention pooling).

Reference computation:
    hid = transpose(hidden,(1,0,2)).reshape(B,-1)          # (B, 1024)
    e   = enc @ We + (hid @ Wh)[:,None] + b                # (B, T)
    e   = e * mask
    a   = softmax(e, axis=1) * mask;  a /= a.sum(1)
    ctx = einsum('bt,bth->bh', a, enc)                     # (B, 1024)

Key identity: the per-batch constant c = hid@Wh + b shifts every *valid*
energy equally, masked entries are zeroed in both the numerator and the
renormalization denominator, and softmax's own Z cancels under the
renormalize — so exp(c) cancels exactly and the output does not depend on
hidden/Wh/b at all:
    ctx[b] = sum_t mask*exp(e_enc) * enc / sum_t mask*exp(e_enc)
(verified vs the jax reference: rel err ~2e-6, pure fp noise).

Device pipeline per enc tile [128t, 1024h] (f32, natural layout):
    DVE : affine_mul_reduce -> p16 = fp16(enc*We), e[:,j] = sum_h (f32)
          (single fused pass; enc f32 tile freed immediately)
    PE  : ctx_psum += w16[t]^T @ p16      (fp16 matmul, 1 HW pass)
then ctx = ctx_psum * (1/S) * (1/We) — dividing the *product*-weighted sum
by We recovers the enc-weighted sum (error ~3e-4 from fp16 rounding of p).

Sharding: batch B=32 across 8 cores (4 batches/core); We replicated.
Host precomputes 1/We and the transposed mask layout (tiny arrays).
"""

import numpy as np

N_CORES = 8
B, T, HE = 32, 2048, 1024
B_LOC = B // N_CORES          # 4 batches per core
TT = 128                      # t-tile (partition dim)
NT = T // TT                  # 16 t-tiles per batch
NH = 512                      # matmul free-dim limit (one PSUM bank of f32)

_CACHE = {}


def _build_nc():
    import concourse.bacc as bacc
    import concourse.tile as tile
    from concourse import mybir

    f32 = mybir.dt.float32
    f16 = mybir.dt.float16
    Exp = mybir.ActivationFunctionType.Exp

    nc = bacc.Bacc("TRN2")
    enc = nc.dram_tensor("enc", [B_LOC, T, HE], f32, kind="ExternalInput")
    mskt = nc.dram_tensor("mskt", [128, B_LOC * NT], f32, kind="ExternalInput")
    we = nc.dram_tensor("we", [1, HE], f32, kind="ExternalInput")
    invwe = nc.dram_tensor("invwe", [1, HE], f32, kind="ExternalInput")
    out = nc.dram_tensor("out", [B_LOC, HE], f32, kind="ExternalOutput")

    with tile.TileContext(nc) as tc:
        with (
            tc.tile_pool(name="singles", bufs=1) as singles,
            tc.tile_pool(name="encp", bufs=12) as encp,
            tc.tile_pool(name="p16p", bufs=20) as p16p,
            tc.tile_pool(name="stats", bufs=4) as stats,
            tc.tile_pool(name="ctxp", bufs=2, space="PSUM") as ctxp,
            tc.tile_pool(name="spsum", bufs=2, space="PSUM") as spsum,
        ):
            # We broadcast to all 128 partitions (one-time, 512KB)
            we_b = singles.tile([128, HE], f32, tag="we_b")
            nc.gpsimd.dma_start(out=we_b, in_=we[0:1, :].partition_broadcast(128))

            inv_sb = singles.tile([1, HE], f32, tag="invwe")
            nc.sync.dma_start(out=inv_sb, in_=invwe[0:1, :])

            ones_col = singles.tile([128, 1], f32, tag="ones")
            nc.vector.memset(ones_col, 1.0)

            # transposed mask [t-within-tile, (b, tile)] — one natural DMA
            mask_all = singles.tile([128, B_LOC * NT], f32, tag="mask")
            nc.sync.dma_start(out=mask_all, in_=mskt[:, :])

            for b in range(B_LOC):
                mb = mask_all[:, b * NT : (b + 1) * NT]
                e_b = stats.tile([128, NT], f32, tag="e_b")
                p16_tiles = []
                for j in range(NT):
                    et = encp.tile([128, HE], f32, tag="enc_t")
                    nc.sync.dma_start(out=et, in_=enc[b, j * TT : (j + 1) * TT, :])
                    p16 = p16p.tile([128, HE], f16, tag="p16")
                    p16_tiles.append(p16)
                    # p16 = fp16(enc * We); e_b[:, j] = sum_h enc*We (f32)
                    nc.vector.affine_mul_reduce(
                        out=p16,
                        accum_out=e_b[:, j : j + 1],
                        in0=et,
                        in1=we_b,
                        scale=1.0,
                        bias=0.0,
                    )

                # w = mask * exp(e * mask); ws[p] = sum_j w[p, j]
                masked = stats.tile([128, NT], f32, tag="masked")
                nc.vector.tensor_mul(masked, e_b, mb)
                expd = stats.tile([128, NT], f32, tag="expd")
                nc.scalar.activation(expd, masked, Exp)
                w_b = stats.tile([128, NT], f32, tag="w_b")
                ws = stats.tile([128, 1], f32, tag="ws")
                nc.vector.tensor_mul(w_b, expd, mb)
                nc.vector.reduce_sum(ws, w_b, axis=mybir.AxisListType.X)
                w16 = stats.tile([128, NT], f16, tag="w16")
                nc.vector.tensor_copy(w16, w_b)

                # S = sum_p ws[p]  (partition reduce via PE), recip = 1/S
                s_ps = spsum.tile([1, 1], f32, tag="s_ps")
                nc.tensor.matmul(s_ps, ws, ones_col, start=True, stop=True)
                recip = stats.tile([1, 1], f32, tag="recip")
                nc.vector.reciprocal(recip, s_ps)

                # ctxP[h] = sum_t w16[t] * p16[t, h], accumulated over t-tiles
                ctx = ctxp.tile([1, 2, NH], f32, tag="ctx")
                for j in range(NT):
                    for h in range(2):
                        nc.tensor.matmul(
                            ctx[:, h, :],
                            w16[:, j : j + 1],
                            p16_tiles[j][:, h * NH : (h + 1) * NH],
                            start=(j == 0),
                            stop=(j == NT - 1),
                        )

                # out[b] = (ctxP * (1/S)) * (1/We)   — one fused DVE op
                ctx_sb = stats.tile([1, HE], f32, tag="ctx_sb")
                dummy = stats.tile([1, 1], f32, tag="dummy")
                nc.vector.affine_mul_reduce(
                    out=ctx_sb.rearrange("p (g h) -> p g h", g=2),
                    accum_out=dummy,
                    in0=ctx[:, :, :],
                    in1=inv_sb.rearrange("p (g h) -> p g h", g=2),
                    scale=recip,
                    bias=0.0,
                )
                nc.gpsimd.dma_start(out=out[b : b + 1, :], in_=ctx_sb)

    nc.compile()
    return nc


def _get_nc():
    if "nc" not in _CACHE:
        _CACHE["nc"] = _build_nc()
    return _CACHE["nc"]


def _prep_host_inputs(encoder_outputs, mask, W):
    enc = np.ascontiguousarray(np.asarray(encoder_outputs, dtype=np.float32))
    msk = np.asarray(mask, dtype=np.float32)
    we = np.ascontiguousarray(np.asarray(W, dtype=np.float32)[0:1, HE:])
    invwe = np.ascontiguousarray(1.0 / we)
    return enc, msk, we, invwe


def kernel(hidden, encoder_outputs, mask, W, b):
    from concourse import bass_utils

    # avoid S3 upload attempts if tracing is enabled
    bass_utils.upload_artifacts = lambda tmpdir: f"local:{tmpdir}"

    nc = _get_nc()
    enc, msk, we, invwe = _prep_host_inputs(encoder_outputs, mask, W)

    in_maps = []
    for i in range(N_CORES):
        mloc = msk[i * B_LOC : (i + 1) * B_LOC]               # [4, 2048]
        mskt = np.ascontiguousarray(
            mloc.reshape(B_LOC, NT, TT).transpose(2, 0, 1).reshape(TT, B_LOC * NT)
        )
        in_maps.append(
            {
                "enc": np.ascontiguousarray(enc[i * B_LOC : (i + 1) * B_LOC]),
                "mskt": mskt,
                "we": we,
                "invwe": invwe,
            }
        )

    def _run():
        return bass_utils.run_bass_kernel_spmd(
            nc, in_maps, core_ids=list(range(N_CORES))
        )

    try:
        res = _run()
    except Exception:
        # transient device-state failures have been observed; retry once
        res = _run()
    _CACHE["last_results"] = res
    return np.concatenate([r["out"] for r in res.results], axis=0)
